# revision 1
# baseline (speedup 1.0000x reference)
"""Trainium2 Bass kernel for nn_BasicBlock (sparse-conv gather-GEMM block, 8 cores).

Computation (reference):
    h1 = sum_k mask1[k,n] * x[kmap1[k,n]] @ W1[k]
    o1 = relu(bn(h1))
    h2 = sum_k mask2[k,n] * o1[kmap2[k,n]] @ W2[k]
    out = relu(bn(h2) + x)

Mapping: voxel dim sharded 8 ways (25000 rows/core). Masks are folded into the
kernel maps on the host (masked entries point at an all-zero table row). Each
core gathers rows from a replicated table with indirect DMAs (128 rows/call),
transposes gathered tiles on the TensorEngine (channels -> partitions), and
accumulates the 27 W_k matmuls in PSUM. BN stats are all-reduced across cores;
the normalized conv1 output is all-gathered so conv2 can gather across shard
boundaries. Output rows return per-core and are concatenated on the host.
"""
import math
from contextlib import ExitStack

import numpy as np

N_GLOB = 200000
C = 128
K = 27
N_CORES = 8
EPS = 1e-5

R = N_GLOB // N_CORES          # 25000 valid rows per core
TILES = math.ceil(R / 128)     # 196
R_PAD = TILES * 128            # 25088
SUP = 4                        # 128-row sub-tiles per super-tile
NSUP = math.ceil(TILES / SUP)  # 49
TABLE_ROWS = ((N_GLOB + 128 + 127) // 128) * 128 + 64  # 200256; rows >= N_GLOB are zero
ZROW = N_GLOB                  # index of an all-zero row

_TRACE = False
_TMPDIR = None
LAST_RESULTS = None

_NC_CACHE = {}


def _build(tiles=TILES, r_valid=R, table_rows=TABLE_ROWS, n_glob=N_GLOB,
           n_cores=N_CORES, sup=SUP):
    from concourse import bass, bacc, tile, mybir

    f32 = mybir.dt.float32
    f32r = mybir.dt.float32r
    i32 = mybir.dt.int32
    AF = mybir.ActivationFunctionType
    ALU = mybir.AluOpType
    AX = mybir.AxisListType

    nsup = math.ceil(tiles / sup)
    r_pad = tiles * 128
    rg = [list(range(n_cores))]

    nc = bacc.Bacc("TRN2", target_bir_lowering=False, debug=False,
                   num_devices=n_cores)

    x_aug = nc.dram_tensor("x_aug", [table_rows, C], f32, kind="ExternalInput").ap()
    idx1 = nc.dram_tensor("idx1", [nsup, 128, sup * K], i32, kind="ExternalInput").ap()
    idx2 = nc.dram_tensor("idx2", [nsup, 128, sup * K], i32, kind="ExternalInput").ap()
    w1d = nc.dram_tensor("w1", [K, C, C], f32, kind="ExternalInput").ap()
    w2d = nc.dram_tensor("w2", [K, C, C], f32, kind="ExternalInput").ap()
    bnt = nc.dram_tensor("bnt", [C, 4], f32, kind="ExternalInput").ap()
    xres = nc.dram_tensor("xres", [r_pad, C], f32, kind="ExternalInput").ap()
    y = nc.dram_tensor("y", [r_pad, C], f32, kind="ExternalOutput").ap()

    with tile.TileContext(nc) as tc, ExitStack() as ctx:
        dram = ctx.enter_context(tc.tile_pool(name="dram", bufs=1, space="DRAM"))
        cc_in = dram.tile([r_valid, C], f32, tag="cc_in")
        cc_out = dram.tile([table_rows, C], f32, tag="cc_out")
        ccs_in = dram.tile([C, 2], f32, tag="ccs_in")
        ccs_out = dram.tile([C, 2], f32, tag="ccs_out")
        ccs2_in = dram.tile([C, 2], f32, tag="ccs2_in")
        ccs2_out = dram.tile([C, 2], f32, tag="ccs2_out")

        wpool = ctx.enter_context(tc.tile_pool(name="wpool", bufs=1))
        hpool = ctx.enter_context(tc.tile_pool(name="hpool", bufs=1))
        spool = ctx.enter_context(tc.tile_pool(name="spool", bufs=1))
        ipool = ctx.enter_context(tc.tile_pool(name="ipool", bufs=4))
        gpool = ctx.enter_context(tc.tile_pool(name="gpool", bufs=20))
        gtpool = ctx.enter_context(tc.tile_pool(name="gtpool", bufs=6))
        sqpool = ctx.enter_context(tc.tile_pool(name="sqpool", bufs=2))
        opool = ctx.enter_context(tc.tile_pool(name="opool", bufs=2))
        ptp = ctx.enter_context(tc.tile_pool(name="ptp", bufs=2, space="PSUM"))
        php = ctx.enter_context(tc.tile_pool(name="php", bufs=2, space="PSUM"))

        # --- resident constants ---
        from concourse.masks import make_identity
        ident = wpool.tile([128, 128], f32, tag="ident")
        make_identity(nc, ident[:])

        w1s = wpool.tile([128, K * C], f32r, tag="w1")
        w2s = wpool.tile([128, K * C], f32r, tag="w2")
        wstage = wpool.tile([128, K * C], f32, tag="wstage")
        nc.sync.dma_start(out=wstage[:].rearrange("p (k c) -> p k c", c=C),
                          in_=w1d.rearrange("k ci co -> ci k co"))
        nc.vector.tensor_copy(out=w1s[:], in_=wstage[:])
        wstage2 = wpool.tile([128, K * C], f32, tag="wstage")
        nc.sync.dma_start(out=wstage2[:].rearrange("p (k c) -> p k c", c=C),
                          in_=w2d.rearrange("k ci co -> ci k co"))
        nc.vector.tensor_copy(out=w2s[:], in_=wstage2[:])
        bns = wpool.tile([128, 4], f32, tag="bns")
        nc.sync.dma_start(out=bns[:], in_=bnt[:, :])

        # zero the tail rows of cc_out (the masked-entry zero rows)
        zt = wpool.tile([128, C], f32, tag="zt")
        nc.vector.memset(zt[:], 0.0)
        off = n_glob
        while off < table_rows:
            n = min(128, table_rows - off)
            nc.sync.dma_start(out=cc_out[off:off + n, :], in_=zt[:n, :])
            off += n

        st_sum1 = spool.tile([128, nsup], f32, tag="st_sum1")
        st_sq1 = spool.tile([128, nsup], f32, tag="st_sq1")
        st_sum2 = spool.tile([128, nsup], f32, tag="st_sum2")
        st_sq2 = spool.tile([128, nsup], f32, tag="st_sq2")

        def conv(src_ap, idx_ap, w_sb, h_sb, st_sum, st_sq):
            for T in range(nsup):
                subs = min(sup, tiles - T * sup)
                w_cols = subs * 128
                it = ipool.tile([128, sup * K], i32, tag="it")
                nc.sync.dma_start(out=it[:], in_=idx_ap[T, :, :])
                gs = {}
                for k in range(K):
                    for s in range(subs):
                        g = gpool.tile([128, C], f32, tag="g")
                        j = k * sup + s
                        nc.gpsimd.indirect_dma_start(
                            out=g[:], out_offset=None, in_=src_ap,
                            in_offset=bass.IndirectOffsetOnAxis(
                                ap=it[:, j:j + 1], axis=0))
                        gs[(k, s)] = g
                ph = php.tile([128, sup * 128], f32, tag="ph")
                for k in range(K):
                    pt = ptp.tile([128, sup * 128], f32, tag="pt")
                    for s in range(subs):
                        nc.tensor.transpose(
                            out=pt[:, s * 128:(s + 1) * 128],
                            in_=gs[(k, s)][:], identity=ident[:])
                    gt = gtpool.tile([128, sup * 128], f32r, tag="gt")
                    nc.vector.tensor_copy(out=gt[:, :w_cols], in_=pt[:, :w_cols])
                    nc.tensor.matmul(ph[:, :w_cols],
                                     lhsT=w_sb[:, k * C:(k + 1) * C],
                                     rhs=gt[:, :w_cols],
                                     start=(k == 0), stop=(k == K - 1))
                hs = h_sb[:, T * sup * 128: T * sup * 128 + w_cols]
                nc.vector.tensor_copy(out=hs, in_=ph[:, :w_cols])
                nc.vector.tensor_reduce(out=st_sum[:, T:T + 1], in_=hs,
                                        axis=AX.X, op=ALU.add)
                sq = sqpool.tile([128, sup * 128], f32, tag="sq")
                nc.scalar.activation(out=sq[:, :w_cols], in_=hs, func=AF.Square,
                                     accum_out=st_sq[:, T:T + 1])

        def bn_coeffs(st_sum, st_sq, gcol, bcol, cin, cout, name):
            ssum = spool.tile([128, 2], f32, tag=f"pk{name}")
            nc.vector.tensor_reduce(out=ssum[:, 0:1], in_=st_sum[:, :nsup],
                                    axis=AX.X, op=ALU.add)
            nc.vector.tensor_reduce(out=ssum[:, 1:2], in_=st_sq[:, :nsup],
                                    axis=AX.X, op=ALU.add)
            nc.sync.dma_start(out=cin[:, :], in_=ssum[:])
            nc.gpsimd.collective_compute(
                "AllReduce", ALU.add, replica_groups=rg,
                ins=[cin.opt()], outs=[cout.opt()])
            g = spool.tile([128, 2], f32, tag=f"gs{name}")
            nc.sync.dma_start(out=g[:], in_=cout[:, :])
            w = spool.tile([128, 6], f32, tag=f"wk{name}")
            mu, ex2, var = w[:, 0:1], w[:, 1:2], w[:, 2:3]
            nc.vector.tensor_scalar_mul(mu, g[:, 0:1], 1.0 / n_glob)
            nc.vector.tensor_scalar_mul(ex2, g[:, 1:2], 1.0 / n_glob)
            nc.vector.tensor_tensor(out=var, in0=mu, in1=mu, op=ALU.mult)
            nc.vector.tensor_tensor(out=var, in0=ex2, in1=var, op=ALU.subtract)
            nc.vector.tensor_scalar_add(var, var, EPS)
            sd = w[:, 3:4]
            nc.scalar.sqrt(out=sd, in_=var)
            rstd = w[:, 4:5]
            nc.vector.reciprocal(out=rstd, in_=sd)
            ab = spool.tile([128, 2], f32, tag=f"ab{name}")
            a, b = ab[:, 0:1], ab[:, 1:2]
            nc.vector.tensor_tensor(out=a, in0=rstd, in1=bns[:, gcol:gcol + 1],
                                    op=ALU.mult)
            t = w[:, 5:6]
            nc.vector.tensor_tensor(out=t, in0=mu, in1=a, op=ALU.mult)
            nc.vector.tensor_tensor(out=b, in0=bns[:, bcol:bcol + 1], in1=t,
                                    op=ALU.subtract)
            return a, b

        # ---- conv1 ----
        h1 = hpool.tile([128, r_pad], f32, tag="h")
        conv(x_aug, idx1, w1s, h1, st_sum1, st_sq1)
        a1, b1 = bn_coeffs(st_sum1, st_sq1, 0, 1, ccs_in, ccs_out, "1")

        # ---- bn1 + relu + transpose back to row-major + allgather ----
        for T in range(nsup):
            subs = min(sup, tiles - T * sup)
            w_cols = subs * 128
            hs = h1[:, T * sup * 128: T * sup * 128 + w_cols]
            o = opool.tile([128, sup * 128], f32, tag="o")
            nc.scalar.activation(out=o[:, :w_cols], in_=hs, func=AF.Relu,
                                 bias=b1, scale=a1)
            pt = ptp.tile([128, sup * 128], f32, tag="pt")
            for s in range(subs):
                nc.tensor.transpose(
                    out=pt[:, s * 128:(s + 1) * 128],
                    in_=o[:, s * 128:(s + 1) * 128], identity=ident[:])
            orow = opool.tile([128, sup * 128], f32, tag="orow")
            nc.vector.tensor_copy(out=orow[:, :w_cols], in_=pt[:, :w_cols])
            for s in range(subs):
                r0 = T * sup * 128 + s * 128
                nrows = max(0, min(128, r_valid - r0))
                if nrows:
                    nc.sync.dma_start(out=cc_in[r0:r0 + nrows, :],
                                      in_=orow[:nrows, s * 128:s * 128 + 128])
        nc.gpsimd.collective_compute(
            "AllGather", ALU.bypass, replica_groups=rg,
            ins=[cc_in.opt()], outs=[cc_out[0:n_glob, :].opt()])

        # ---- conv2 (gathers from the all-gathered o1 table) ----
        h2 = hpool.tile([128, r_pad], f32, tag="h")
        conv(cc_out[:, :], idx2, w2s, h2, st_sum2, st_sq2)
        a2, b2 = bn_coeffs(st_sum2, st_sq2, 2, 3, ccs2_in, ccs2_out, "2")

        # ---- bn2 + residual + relu -> output rows ----
        for T in range(nsup):
            subs = min(sup, tiles - T * sup)
            w_cols = subs * 128
            r0 = T * sup * 128
            hs = h2[:, r0: r0 + w_cols]
            o = opool.tile([128, sup * 128], f32, tag="o")
            nc.scalar.activation(out=o[:, :w_cols], in_=hs, func=AF.Identity,
                                 bias=b2, scale=a2)
            pt = ptp.tile([128, sup * 128], f32, tag="pt")
            for s in range(subs):
                nc.tensor.transpose(
                    out=pt[:, s * 128:(s + 1) * 128],
                    in_=o[:, s * 128:(s + 1) * 128], identity=ident[:])
            rrow = opool.tile([128, sup * 128], f32, tag="orow")
            nc.vector.tensor_copy(out=rrow[:, :w_cols], in_=pt[:, :w_cols])
            xr = opool.tile([128, sup * 128], f32, tag="xr")
            nc.sync.dma_start(
                out=xr[:].rearrange("p (s c) -> p s c", c=C)[:, :subs, :],
                in_=xres[r0:r0 + w_cols, :].rearrange("(s p) c -> p s c", p=128))
            nc.vector.tensor_tensor(out=rrow[:, :w_cols], in0=rrow[:, :w_cols],
                                    in1=xr[:, :w_cols], op=ALU.add)
            yt = opool.tile([128, sup * 128], f32, tag="yt")
            nc.scalar.activation(out=yt[:, :w_cols], in_=rrow[:, :w_cols],
                                 func=AF.Relu)
            nc.sync.dma_start(
                out=y[r0:r0 + w_cols, :].rearrange("(s p) c -> p s c", p=128),
                in_=yt[:].rearrange("p (s c) -> p s c", c=C)[:, :subs, :])

    nc.compile()
    return nc


def _prep_idx(kmap, mask, tiles, r_valid, n_glob, n_cores, sup, zrow):
    """Per-core index planes: idx[c][T, p, k*sup+s] = eff_kmap[k, base+T*sup*128+s*128+p]."""
    k = kmap.shape[0]
    nsup = math.ceil(tiles / sup)
    r_pad = tiles * 128
    eff = np.where(mask != 0, kmap, zrow).astype(np.int32)
    out = []
    for c in range(n_cores):
        base = c * r_valid
        slab = eff[:, base:base + r_valid]
        if r_pad > r_valid:
            pad = np.full((k, r_pad - r_valid), zrow, np.int32)
            slab = np.concatenate([slab, pad], axis=1)
        if nsup * sup * 128 > r_pad:
            pad = np.full((k, nsup * sup * 128 - r_pad), zrow, np.int32)
            slab = np.concatenate([slab, pad], axis=1)
        s4 = slab.reshape(k, nsup, sup, 128)
        out.append(np.ascontiguousarray(
            s4.transpose(1, 3, 0, 2).reshape(nsup, 128, k * sup)))
    return out


def kernel(x, W1, gamma1, beta1, W2, gamma2, beta2, kmap1, kmap2, mask1, mask2):
    from concourse import bass_utils
    global LAST_RESULTS

    x = np.asarray(x, np.float32)
    x_aug = np.zeros((TABLE_ROWS, C), np.float32)
    x_aug[:N_GLOB] = x

    idx1 = _prep_idx(np.asarray(kmap1), np.asarray(mask1), TILES, R, N_GLOB,
                     N_CORES, SUP, ZROW)
    idx2 = _prep_idx(np.asarray(kmap2), np.asarray(mask2), TILES, R, N_GLOB,
                     N_CORES, SUP, ZROW)
    bnt = np.stack([np.asarray(gamma1, np.float32), np.asarray(beta1, np.float32),
                    np.asarray(gamma2, np.float32), np.asarray(beta2, np.float32)],
                   axis=1)
    w1 = np.ascontiguousarray(np.asarray(W1, np.float32))
    w2 = np.ascontiguousarray(np.asarray(W2, np.float32))

    if "full" not in _NC_CACHE:
        _NC_CACHE["full"] = _build()
    nc = _NC_CACHE["full"]

    in_maps = []
    for c in range(N_CORES):
        base = c * R
        in_maps.append({
            "x_aug": x_aug,
            "idx1": idx1[c],
            "idx2": idx2[c],
            "w1": w1,
            "w2": w2,
            "bnt": bnt,
            "xres": np.ascontiguousarray(x_aug[base:base + R_PAD]),
        })

    kwargs = {}
    if _TRACE:
        kwargs = dict(trace=True, tmpdir=_TMPDIR)
    res = bass_utils.run_bass_kernel_spmd(
        nc, in_maps, core_ids=list(range(N_CORES)), **kwargs)
    LAST_RESULTS = res
    out = np.concatenate([res.results[c]["y"][:R] for c in range(N_CORES)], axis=0)
    return np.ascontiguousarray(out, dtype=np.float32)



# revision 9
# speedup vs baseline: 1.3070x; 1.3070x over previous
"""Trainium2 Bass kernel for nn_BasicBlock — windowed dma_gather + scatter-add design.

Computation (reference):
    h1 = sum_k mask1[k,n] * x[kmap1[k,n]] @ W1[k]
    o1 = relu(bn(h1))
    h2 = sum_k mask2[k,n] * o1[kmap2[k,n]] @ W2[k]
    out = relu(bn(h2) + x)

Voxels sharded 8 ways (25000/core); the feature table (x for conv1, the
all-gathered normalized o1 for conv2) is replicated in DRAM as bf16.

The old per-128-row indirect-DMA gather costs ~1.4us of Pool-sequencer time
per call (measured), an ~15ms floor for 1.35M rows. Instead, each conv is
reorganized around InstDMAGatherAnt/InstDMAScatterAddAnt, whose int16 index
limit is met by splitting the 200256-row table into 7 windows of 32768 rows:

  for each window w, k-triple: one dma_gather(transpose=True) fetches ALL
  live slots (mask=1) whose source row falls in w — compact, int16
  window-local indices, output already transposed [ci, slots];
  per k: W_k^T @ compact -> PSUM; PE-transposes back to row-major;
  one dma_scatter_add adds the rows into h-slab[k%4] at their voxel ids
  (int16, < 25088). Masked slots are never gathered (halves traffic).

h = sum of 4 slabs, computed during a streaming stats pass (transposed into
SBUF); BN stats all-reduced; bn1+relu rows all-gathered (bf16) for conv2;
bn2 + f32 residual + relu written per-core and concatenated on the host.
Index lists are padded to static shapes (gather pad -> window row 0,
scatter pad -> -1 which the ucode ignores), so the program is input-shape
independent; counts are asserted on the host.
"""
import math
from contextlib import ExitStack

import numpy as np

N_GLOB = 200000
C = 128
K = 27
N_CORES = 8
EPS = 1e-5

R = N_GLOB // N_CORES          # 25000 valid rows per core
TILES = math.ceil(R / 128)     # 196
R_PAD = TILES * 128            # 25088
SUP = 4
NSUP = math.ceil(TILES / SUP)  # 49
TABLE_ROWS = ((N_GLOB + 128 + 127) // 128) * 128 + 64  # 200256
ZROW = N_GLOB

WROWS = 28672                  # gather window rows (int16-addressable)
NW = 7                         # uniform windows (last covers 28224 table rows)
S = 2048                       # padded slots per (w, k); gathers issued in 512-chunks
GCH = 512                      # max idx per dma_gather/scatter call (hw ring limit)
NSLAB = 4                      # h accumulation slabs (k % 4)
SC_ONE = True                  # one 2048-row scatter per (w,k) (else 512-chunks)

_TRACE = False
_TMPDIR = None
LAST_RESULTS = None

_NC_CACHE = {}


def _build():
    from concourse import bass, bacc, tile, mybir

    f32 = mybir.dt.float32
    bf16 = mybir.dt.bfloat16
    i16 = mybir.dt.int16
    AF = mybir.ActivationFunctionType
    ALU = mybir.AluOpType
    AX = mybir.AxisListType

    r_pad = R_PAD
    nsup = NSUP
    rg = [list(range(N_CORES))]

    nc = bacc.Bacc("TRN2", target_bir_lowering=False, debug=False,
                   num_devices=N_CORES)

    x_aug = nc.dram_tensor("x_aug", [TABLE_ROWS, C], bf16, kind="ExternalInput").ap()
    # idx planes: [w, k, 128, S/16] (wrap16 layout)
    g1b = nc.dram_tensor("g1b", [NW, K, 128, S // 16], i16, kind="ExternalInput").ap()
    s1b = nc.dram_tensor("s1b", [NW, K, 128, S // 16], i16, kind="ExternalInput").ap()
    g2b = nc.dram_tensor("g2b", [NW, K, 128, S // 16], i16, kind="ExternalInput").ap()
    s2b = nc.dram_tensor("s2b", [NW, K, 128, S // 16], i16, kind="ExternalInput").ap()
    w1d = nc.dram_tensor("w1", [K, C, C], bf16, kind="ExternalInput").ap()
    w2d = nc.dram_tensor("w2", [K, C, C], bf16, kind="ExternalInput").ap()
    bnt = nc.dram_tensor("bnt", [C, 4], f32, kind="ExternalInput").ap()
    xres = nc.dram_tensor("xres", [r_pad, C], f32, kind="ExternalInput").ap()
    y = nc.dram_tensor("y", [r_pad, C], f32, kind="ExternalOutput").ap()

    with tile.TileContext(nc) as tc, ExitStack() as ctx:
        dram = ctx.enter_context(tc.tile_pool(name="dram", bufs=1, space="DRAM"))
        cc_in = dram.tile([R, C], bf16, tag="cc_in")
        cc_out = dram.tile([TABLE_ROWS, C], bf16, tag="cc_out")
        ccs_in = dram.tile([C, 2], f32, tag="ccs_in")
        ccs_out = dram.tile([C, 2], f32, tag="ccs_out")
        ccs2_in = dram.tile([C, 2], f32, tag="ccs2_in")
        ccs2_out = dram.tile([C, 2], f32, tag="ccs2_out")
        slabs1 = [dram.tile([r_pad + 128, C], bf16, tag=f"hs1_{i}", name=f"hs1_{i}")
                  for i in range(NSLAB)]
        slabs2 = [dram.tile([r_pad + 128, C], bf16, tag=f"hs2_{i}", name=f"hs2_{i}")
                  for i in range(NSLAB)]

        wpool = ctx.enter_context(tc.tile_pool(name="wpool", bufs=1))
        hpool = ctx.enter_context(tc.tile_pool(name="hpool", bufs=1))
        spool = ctx.enter_context(tc.tile_pool(name="spool", bufs=1))
        gipool = ctx.enter_context(tc.tile_pool(name="gipool", bufs=3))
        sipool = ctx.enter_context(tc.tile_pool(name="sipool", bufs=6))
        gcpool = ctx.enter_context(tc.tile_pool(name="gcpool", bufs=2))
        sdpool = ctx.enter_context(tc.tile_pool(name="sdpool", bufs=4))
        gtpool = ctx.enter_context(tc.tile_pool(name="gtpool", bufs=6))
        rpool = ctx.enter_context(tc.tile_pool(name="rpool", bufs=2))
        opool = ctx.enter_context(tc.tile_pool(name="opool", bufs=2))
        pmp = ctx.enter_context(tc.tile_pool(name="pmp", bufs=3, space="PSUM"))
        ptp = ctx.enter_context(tc.tile_pool(name="ptp", bufs=4, space="PSUM"))

        # --- resident constants ---
        from concourse.masks import make_identity
        identf = wpool.tile([128, 128], f32, tag="identf")
        make_identity(nc, identf[:])
        ident = wpool.tile([128, 128], bf16, tag="ident")
        nc.vector.tensor_copy(out=ident[:], in_=identf[:])

        w1s = wpool.tile([128, K * C], bf16, tag="w1")
        w2s = wpool.tile([128, K * C], bf16, tag="w2")
        nc.sync.dma_start(out=w1s[:].rearrange("p (k c) -> p k c", c=C),
                          in_=w1d.rearrange("k ci co -> ci k co"))
        nc.sync.dma_start(out=w2s[:].rearrange("p (k c) -> p k c", c=C),
                          in_=w2d.rearrange("k ci co -> ci k co"))
        bns = wpool.tile([128, 4], f32, tag="bns")
        nc.sync.dma_start(out=bns[:], in_=bnt[:, :])

        zt = wpool.tile([128, 8 * C], bf16, tag="zt")
        nc.vector.memset(zt[:], 0.0)

        def zero_slabs(slabs):
            for sl in slabs:
                off = 0
                while off < r_pad:
                    n = min(1024, r_pad - off)
                    nc.sync.dma_start(
                        out=sl[off:off + n, :].rearrange("(b p) c -> p b c", p=128),
                        in_=zt[:].rearrange("p (b c) -> p b c", c=C)[:, :n // 128, :])
                    off += n

        # zero the tail rows of cc_out (mask-skipped rows never read; keep clean)
        off = N_GLOB
        while off < TABLE_ROWS:
            n = min(1024, TABLE_ROWS - off)
            nc.sync.dma_start(
                out=cc_out[off:off + n, :].rearrange("(b p) c -> p b c", p=128),
                in_=zt[:].rearrange("p (b c) -> p b c", c=C)[:, :n // 128, :])
            off += n

        st_sum1 = spool.tile([128, nsup], f32, tag="st_sum1")
        st_sq1 = spool.tile([128, nsup], f32, tag="st_sq1")
        st_sum2 = spool.tile([128, nsup], f32, tag="st_sum2")
        st_sq2 = spool.tile([128, nsup], f32, tag="st_sq2")

        def conv(src_ap, gb, sb, w_sb, slabs):
            for w in range(NW):
                wbase = w * WROWS
                wrows = min(WROWS, TABLE_ROWS - wbase)
                for k in range(K):
                    git = gipool.tile([128, S // 16], i16, tag="git")
                    nc.sync.dma_start(out=git[:], in_=gb[w, k, :, :])
                    sit = sipool.tile([128, S // 16], i16, tag="sit")
                    nc.sync.dma_start(out=sit[:], in_=sb[w, k, :, :])
                    gc = gcpool.tile([128, S], bf16, tag="gc")
                    for c0 in range(0, S, GCH):
                        nc.gpsimd.dma_gather(
                            out_ap=gc[:, c0:c0 + GCH].rearrange(
                                "p (a n) -> p a n", a=1),
                            in_ap=src_ap[wbase:wbase + wrows, :],
                            idxs_ap=git[:, c0 // 16:(c0 + GCH) // 16],
                            num_idxs=GCH, num_idxs_reg=GCH,
                            elem_size=C, transpose=True)
                    sd = sdpool.tile([128, S], bf16, tag="sd")
                    for g in range(S // 512):
                        pm = pmp.tile([128, 512], f32, tag="pm")
                        nc.tensor.matmul(
                            pm[:], lhsT=w_sb[:, k * C:(k + 1) * C],
                            rhs=gc[:, g * 512:(g + 1) * 512],
                            start=True, stop=True)
                        gt = gtpool.tile([128, 512], bf16, tag="gt")
                        if g % 2 == 0:
                            nc.vector.tensor_copy(out=gt[:], in_=pm[:])
                        else:
                            nc.scalar.activation(out=gt[:], in_=pm[:],
                                                 func=AF.Identity)
                        pt = ptp.tile([128, 512], bf16, tag="pt")
                        for s in range(4):
                            nc.tensor.transpose(
                                out=pt[:, s * 128:(s + 1) * 128],
                                in_=gt[:, s * 128:(s + 1) * 128],
                                identity=ident[:])
                        if g % 2 == 0:
                            nc.scalar.activation(out=sd[:, g * 512:(g + 1) * 512],
                                                 in_=pt[:], func=AF.Identity)
                        else:
                            nc.vector.tensor_copy(out=sd[:, g * 512:(g + 1) * 512],
                                                  in_=pt[:])
                    if SC_ONE:
                        nc.gpsimd.dma_scatter_add(
                            out_ap=slabs[k % NSLAB][:, :],
                            in_ap=sd[:].rearrange("p (b c) -> p b c", c=C),
                            idxs_ap=sit[:],
                            num_idxs=S, num_idxs_reg=S,
                            elem_size=C)
                    else:
                        for g in range(S // GCH):
                            nc.gpsimd.dma_scatter_add(
                                out_ap=slabs[k % NSLAB][:, :],
                                in_ap=sd[:, g * GCH:(g + 1) * GCH].rearrange(
                                    "p (b c) -> p b c", c=C),
                                idxs_ap=sit[:, g * GCH // 16:(g + 1) * GCH // 16],
                                num_idxs=GCH, num_idxs_reg=GCH,
                                elem_size=C)

        def stats(slabs, h_sb, st_sum, st_sq):
            for t in range(nsup):
                r0 = t * SUP * 128
                acc = None
                parts = []
                for i in range(NSLAB):
                    rt = rpool.tile([128, SUP * C], bf16, tag=f"rt{i}")
                    nc.sync.dma_start(
                        out=rt[:].rearrange("p (b c) -> p b c", c=C),
                        in_=slabs[i][r0:r0 + SUP * 128, :].rearrange(
                            "(b p) c -> p b c", p=128))
                    parts.append(rt)
                s01 = rpool.tile([128, SUP * C], bf16, tag="s01")
                nc.vector.tensor_tensor(out=s01[:], in0=parts[0][:], in1=parts[1][:],
                                        op=ALU.add)
                s23 = rpool.tile([128, SUP * C], bf16, tag="s23")
                nc.vector.tensor_tensor(out=s23[:], in0=parts[2][:], in1=parts[3][:],
                                        op=ALU.add)
                hsum = rpool.tile([128, SUP * C], bf16, tag="hsum")
                nc.vector.tensor_tensor(out=hsum[:], in0=s01[:], in1=s23[:],
                                        op=ALU.add)
                pt = ptp.tile([128, 512], bf16, tag="pt")
                for s in range(SUP):
                    nc.tensor.transpose(out=pt[:, s * 128:(s + 1) * 128],
                                        in_=hsum[:, s * C:(s + 1) * C],
                                        identity=ident[:])
                hs = h_sb[:, r0:r0 + SUP * 128]
                nc.scalar.activation(out=hs, in_=pt[:], func=AF.Identity)
                nc.vector.tensor_reduce(out=st_sum[:, t:t + 1], in_=pt[:],
                                        axis=AX.X, op=ALU.add)
                sq = rpool.tile([128, 512], bf16, tag="sq")
                nc.scalar.activation(out=sq[:], in_=pt[:], func=AF.Square,
                                     accum_out=st_sq[:, t:t + 1])

        def bn_coeffs(st_sum, st_sq, gcol, bcol, cin, cout, name):
            ssum = spool.tile([128, 2], f32, tag=f"pk{name}")
            nc.vector.tensor_reduce(out=ssum[:, 0:1], in_=st_sum[:, :nsup],
                                    axis=AX.X, op=ALU.add)
            nc.vector.tensor_reduce(out=ssum[:, 1:2], in_=st_sq[:, :nsup],
                                    axis=AX.X, op=ALU.add)
            nc.sync.dma_start(out=cin[:, :], in_=ssum[:])
            nc.gpsimd.collective_compute(
                "AllReduce", ALU.add, replica_groups=rg,
                ins=[cin.opt()], outs=[cout.opt()])
            g = spool.tile([128, 2], f32, tag=f"gs{name}")
            nc.sync.dma_start(out=g[:], in_=cout[:, :])
            w = spool.tile([128, 6], f32, tag=f"wk{name}")
            mu, ex2, var = w[:, 0:1], w[:, 1:2], w[:, 2:3]
            nc.vector.tensor_scalar_mul(mu, g[:, 0:1], 1.0 / N_GLOB)
            nc.vector.tensor_scalar_mul(ex2, g[:, 1:2], 1.0 / N_GLOB)
            nc.vector.tensor_tensor(out=var, in0=mu, in1=mu, op=ALU.mult)
            nc.vector.tensor_tensor(out=var, in0=ex2, in1=var, op=ALU.subtract)
            nc.vector.tensor_scalar_add(var, var, EPS)
            sd_ = w[:, 3:4]
            nc.scalar.sqrt(out=sd_, in_=var)
            rstd = w[:, 4:5]
            nc.vector.reciprocal(out=rstd, in_=sd_)
            ab = spool.tile([128, 2], f32, tag=f"ab{name}")
            a, b = ab[:, 0:1], ab[:, 1:2]
            nc.vector.tensor_tensor(out=a, in0=rstd, in1=bns[:, gcol:gcol + 1],
                                    op=ALU.mult)
            t = w[:, 5:6]
            nc.vector.tensor_tensor(out=t, in0=mu, in1=a, op=ALU.mult)
            nc.vector.tensor_tensor(out=b, in0=bns[:, bcol:bcol + 1], in1=t,
                                    op=ALU.subtract)
            return a, b

        # ---- conv1 ----
        zero_slabs(slabs1)
        zero_slabs(slabs2)   # early, overlaps conv1
        conv(x_aug, g1b, s1b, w1s, slabs1)
        h1 = hpool.tile([128, r_pad], bf16, tag="h")
        stats(slabs1, h1, st_sum1, st_sq1)
        a1, b1 = bn_coeffs(st_sum1, st_sq1, 0, 1, ccs_in, ccs_out, "1")

        # ---- bn1 + relu -> row-major -> allgather ----
        for t in range(nsup):
            hs = h1[:, t * 512:(t + 1) * 512]
            o = opool.tile([128, 512], bf16, tag="o")
            nc.scalar.activation(out=o[:], in_=hs, func=AF.Relu,
                                 bias=b1, scale=a1)
            pt = ptp.tile([128, 512], bf16, tag="pt")
            for s in range(SUP):
                nc.tensor.transpose(
                    out=pt[:, s * 128:(s + 1) * 128],
                    in_=o[:, s * 128:(s + 1) * 128], identity=ident[:])
            orow = opool.tile([128, 512], bf16, tag="orow")
            nc.vector.tensor_copy(out=orow[:], in_=pt[:])
            for s in range(SUP):
                r0 = t * 512 + s * 128
                nrows = max(0, min(128, R - r0))
                if nrows:
                    nc.sync.dma_start(out=cc_in[r0:r0 + nrows, :],
                                      in_=orow[:nrows, s * 128:s * 128 + 128])
        nc.gpsimd.collective_compute(
            "AllGather", ALU.bypass, replica_groups=rg,
            ins=[cc_in.opt()], outs=[cc_out[0:N_GLOB, :].opt()])

        # ---- conv2 ----
        conv(cc_out[:, :], g2b, s2b, w2s, slabs2)
        h2 = hpool.tile([128, r_pad], bf16, tag="h2")
        stats(slabs2, h2, st_sum2, st_sq2)
        a2, b2 = bn_coeffs(st_sum2, st_sq2, 2, 3, ccs2_in, ccs2_out, "2")

        # ---- bn2 + residual + relu -> y (f32 path) ----
        for t in range(nsup):
            r0 = t * 512
            hs = h2[:, r0:r0 + 512]
            o = opool.tile([128, 512], bf16, tag="o")
            nc.scalar.activation(out=o[:], in_=hs, func=AF.Identity,
                                 bias=b2, scale=a2)
            pt = ptp.tile([128, 512], bf16, tag="pt")
            for s in range(SUP):
                nc.tensor.transpose(
                    out=pt[:, s * 128:(s + 1) * 128],
                    in_=o[:, s * 128:(s + 1) * 128], identity=ident[:])
            rrow = opool.tile([128, 512], f32, tag="rrow")
            nc.vector.tensor_copy(out=rrow[:], in_=pt[:])
            xr = opool.tile([128, 512], f32, tag="xr")
            nc.sync.dma_start(
                out=xr[:].rearrange("p (s c) -> p s c", c=C),
                in_=xres[r0:r0 + 512, :].rearrange("(s p) c -> p s c", p=128))
            nc.vector.tensor_tensor(out=rrow[:], in0=rrow[:], in1=xr[:],
                                    op=ALU.add)
            yt = opool.tile([128, 512], f32, tag="yt")
            nc.scalar.activation(out=yt[:], in_=rrow[:], func=AF.Relu)
            nc.sync.dma_start(
                out=y[r0:r0 + 512, :].rearrange("(s p) c -> p s c", p=128),
                in_=yt[:].rearrange("p (s c) -> p s c", c=C))

    nc.compile()
    return nc


def _wrap16(vals, s):
    """[s] -> [128, s//16] int16: idx i at partition i%16 (replicated x8), col i//16."""
    plane = np.zeros((16, s // 16), np.int16)
    plane[np.arange(len(vals)) % 16, np.arange(len(vals)) // 16] = vals
    return np.tile(plane, (8, 1))


def _prep_conv_idx(kmap, mask, core):
    """Gather/scatter wrap16 idx planes for one conv, one core.

    Returns (gb [NW,K,128,S/16] window-local gather rows,
             sb [NW,K,128,S/16] scatter voxel ids; pads: gather->0,
             scatter->R_PAD sacrificial row)."""
    base = core * R
    eff = kmap[:, base:base + R].astype(np.int64)      # [K, R]
    live = mask[:, base:base + R] != 0
    gb = np.zeros((NW, K, 128, S // 16), np.int16)
    sb = np.full((NW, K, 128, S // 16), R_PAD, np.int16)
    for k in range(K):
        rows = eff[k]
        lv = live[k]
        wins = rows // WROWS
        for w in range(NW):
            m = lv & (wins == w)
            v = np.nonzero(m)[0]
            loc = (rows[v] - w * WROWS).astype(np.int16)
            assert len(v) <= S, f"count {len(v)} > {S} (w={w}, k={k})"
            gplane = np.zeros(S, np.int16)
            gplane[:len(v)] = loc
            splane = np.full(S, R_PAD, np.int16)
            splane[:len(v)] = v.astype(np.int16)
            gb[w, k] = _wrap16(gplane, S)
            sb[w, k] = _wrap16(splane, S)
    return gb, sb


def kernel(x, W1, gamma1, beta1, W2, gamma2, beta2, kmap1, kmap2, mask1, mask2):
    import ml_dtypes
    from concourse import bass_utils
    global LAST_RESULTS

    bf16 = ml_dtypes.bfloat16
    x = np.asarray(x, np.float32)
    x_aug = np.zeros((TABLE_ROWS, C), np.float32)
    x_aug[:N_GLOB] = x
    x_aug_b = x_aug.astype(bf16)

    kmap1 = np.asarray(kmap1)
    kmap2 = np.asarray(kmap2)
    mask1 = np.asarray(mask1)
    mask2 = np.asarray(mask2)
    bnt = np.stack([np.asarray(gamma1, np.float32), np.asarray(beta1, np.float32),
                    np.asarray(gamma2, np.float32), np.asarray(beta2, np.float32)],
                   axis=1)
    w1 = np.ascontiguousarray(np.asarray(W1, np.float32)).astype(bf16)
    w2 = np.ascontiguousarray(np.asarray(W2, np.float32)).astype(bf16)

    if "v4" not in _NC_CACHE:
        _NC_CACHE["v4"] = _build()
    nc = _NC_CACHE["v4"]

    in_maps = []
    for c in range(N_CORES):
        g1 = _prep_conv_idx(kmap1, mask1, c)
        g2 = _prep_conv_idx(kmap2, mask2, c)
        base = c * R
        in_maps.append({
            "x_aug": x_aug_b,
            "g1b": g1[0], "s1b": g1[1],
            "g2b": g2[0], "s2b": g2[1],
            "w1": w1,
            "w2": w2,
            "bnt": bnt,
            "xres": np.ascontiguousarray(x_aug[base:base + R_PAD]),
        })

    kwargs = {}
    if _TRACE:
        kwargs = dict(trace=True, tmpdir=_TMPDIR)
    res = bass_utils.run_bass_kernel_spmd(
        nc, in_maps, core_ids=list(range(N_CORES)), **kwargs)
    LAST_RESULTS = res
    out = np.concatenate([res.results[c]["y"][:R] for c in range(N_CORES)], axis=0)
    return np.ascontiguousarray(out, dtype=np.float32)
